# revision 2
# baseline (speedup 1.0000x reference)
"""Trainium2 Bass kernel for: sigmoid(rowdot(tanh(x1@W.T+b), tanh(x2@W.T+b))).

Sharding: pure data-parallel over batch across 8 NeuronCores.
Per-core shapes are hardcoded (B=65536 total, 8192 rows/core, D_IN=1024,
D_PROJ=128).

Per-core dataflow (batch tile BT=512 rows):
  1. DMA load x1/x2 natural tiles [128p, 4g, 1024d] (2 MiB each, HWDGE).
  2. PE transpose-mode: x[128b,128d] blocks -> PSUM xT blocks, giving
     xT chunks [128d, 512b] (contraction dim on partitions).
  3. DVE/ACT copy PSUM -> SBUF (alternating engines to balance load).
  4. PE matmul (float32r, N=512): oT[j,b] += Wt_k.T @ xT_k, k=0..7.
  5. ACT: t = tanh(oT + bias)  (bias per-partition, PSUM -> SBUF, fused).
  6. DVE: prod = t1 * t2.
  7. PE: sim[1,512] = ones.T @ prod   (partition reduction).
  8. ACT: out = sigmoid(sim); DMA out.
"""

import numpy as np

import concourse.bacc as bacc
import concourse.mybir as mybir
import concourse.tile as tile
from concourse.bass_utils import run_bass_kernel_spmd

N_CORES = 8
B_TOTAL = 65536
BSH = B_TOTAL // N_CORES  # 8192 rows per core
D_IN = 1024
D_PROJ = 128
P = 128
BT = 512                 # batch tile (matmul moving dim)
G = BT // P              # 4 row-groups of 128 per batch tile
NBT = BSH // BT          # 16 batch tiles per core
KC = D_IN // P           # 8 contraction chunks

F32 = mybir.dt.float32
F32R = mybir.dt.float32r


def _build_module():
    nc = bacc.Bacc("TRN2", target_bir_lowering=False, debug=False)

    x1 = nc.dram_tensor("x1", [BSH, D_IN], F32R, kind="ExternalInput").ap()
    x2 = nc.dram_tensor("x2", [BSH, D_IN], F32R, kind="ExternalInput").ap()
    wt = nc.dram_tensor("wt", [D_IN, D_PROJ], F32R, kind="ExternalInput").ap()
    bias = nc.dram_tensor("bias", [P, 1], F32, kind="ExternalInput").ap()
    ident = nc.dram_tensor("ident", [P, P], F32R, kind="ExternalInput").ap()
    ones = nc.dram_tensor("ones", [P, 1], F32R, kind="ExternalInput").ap()
    out = nc.dram_tensor("out", [BSH], F32, kind="ExternalOutput").ap()

    out2d = out.rearrange("(t n) -> t n", n=BT)  # [NBT, BT]
    x1t = x1.rearrange("(t g p) d -> t p g d", p=P, g=G)  # [NBT, 128, G, D_IN]
    x2t = x2.rearrange("(t g p) d -> t p g d", p=P, g=G)

    with tile.TileContext(nc) as tc:
        with (
            tc.tile_pool(name="consts", bufs=1) as cpool,
            tc.tile_pool(name="xnat", bufs=2) as natpool,
            tc.tile_pool(name="xt", bufs=2) as xtpool,
            tc.tile_pool(name="acts", bufs=2) as apool,
            tc.tile_pool(name="outs", bufs=2) as opool_sb,
            tc.tile_pool(name="ptr", bufs=3, space="PSUM") as trpool,
            tc.tile_pool(name="po", bufs=4, space="PSUM") as opool,
            tc.tile_pool(name="psim", bufs=1, space="PSUM") as simpool,
        ):
            wt_sb = cpool.tile([P, KC, D_PROJ], F32R, tag="wt")
            nc.sync.dma_start(out=wt_sb, in_=wt.rearrange("(k p) j -> p k j", p=P))
            bias_sb = cpool.tile([P, 1], F32, tag="bias")
            nc.sync.dma_start(out=bias_sb, in_=bias)
            ident_sb = cpool.tile([P, P], F32R, tag="ident")
            nc.sync.dma_start(out=ident_sb, in_=ident)
            ones_sb = cpool.tile([P, 1], F32R, tag="ones")
            nc.sync.dma_start(out=ones_sb, in_=ones)

            for bt in range(NBT):
                xn1 = natpool.tile([P, G, D_IN], F32R, tag="xn1")
                nc.sync.dma_start(out=xn1, in_=x1t[bt])
                xn2 = natpool.tile([P, G, D_IN], F32R, tag="xn2")
                nc.sync.dma_start(out=xn2, in_=x2t[bt])

                # Transpose x tiles: xT[k][:, g*128:(g+1)*128] = x_blk(g,k).T
                xts = []
                for tens, xn, tag in ((0, xn1, "xt1"), (1, xn2, "xt2")):
                    xt_sb = xtpool.tile([P, KC, BT], F32R, tag=tag)
                    for k in range(KC):
                        ps = trpool.tile([P, BT], F32R, tag="tr")
                        for g in range(G):
                            nc.tensor.transpose(
                                ps[:, g * P:(g + 1) * P],
                                xn[:, g, k * P:(k + 1) * P],
                                ident_sb,
                            )
                        if (k + tens) % 2 == 0:
                            nc.vector.tensor_copy(xt_sb[:, k, :], ps)
                        else:
                            nc.scalar.copy(xt_sb[:, k, :], ps)
                    xts.append(xt_sb)

                # oT[j, b] = sum_k Wt_k.T @ xT_k   (float32r, N=512)
                ts = []
                for tens in range(2):
                    po = opool.tile([P, BT], F32, tag="po")
                    for k in range(KC):
                        nc.tensor.matmul(
                            po,
                            wt_sb[:, k, :],
                            xts[tens][:, k, :],
                            start=(k == 0),
                            stop=(k == KC - 1),
                        )
                    t_sb = apool.tile([P, BT], F32, tag=f"t{tens}")
                    nc.scalar.activation(
                        t_sb, po, mybir.ActivationFunctionType.Tanh, bias=bias_sb
                    )
                    ts.append(t_sb)

                prod = apool.tile([P, BT], F32R, tag="prod")
                nc.vector.tensor_mul(prod, ts[0], ts[1])

                psim = simpool.tile([P, BT], F32, tag="sim")
                nc.tensor.matmul(
                    psim[0:1, :],
                    ones_sb,
                    prod,
                    start=True,
                    stop=True,
                )
                sim_sb = opool_sb.tile([1, BT], F32, tag="sim_sb")
                nc.scalar.activation(
                    sim_sb, psim[0:1, :], mybir.ActivationFunctionType.Sigmoid
                )
                nc.sync.dma_start(out=out2d[bt:bt + 1, :], in_=sim_sb)

    nc.compile()
    return nc


_NC_CACHE = None


def _get_module():
    global _NC_CACHE
    if _NC_CACHE is None:
        _NC_CACHE = _build_module()
    return _NC_CACHE


def kernel(x1, x2, W, b):
    x1 = np.ascontiguousarray(x1, dtype=np.float32)
    x2 = np.ascontiguousarray(x2, dtype=np.float32)
    wt = np.ascontiguousarray(np.asarray(W, dtype=np.float32).T)
    bias = np.ascontiguousarray(np.asarray(b, dtype=np.float32).reshape(P, 1))
    ident = np.eye(P, dtype=np.float32)
    ones = np.ones((P, 1), dtype=np.float32)

    nc = _get_module()
    in_maps = [
        {
            "x1": x1[i * BSH:(i + 1) * BSH],
            "x2": x2[i * BSH:(i + 1) * BSH],
            "wt": wt,
            "bias": bias,
            "ident": ident,
            "ones": ones,
        }
        for i in range(N_CORES)
    ]
    res = run_bass_kernel_spmd(nc, in_maps, core_ids=list(range(N_CORES)))
    return np.concatenate([res.results[i]["out"] for i in range(N_CORES)])


# revision 5
# speedup vs baseline: 1.0639x; 1.0639x over previous
"""Trainium2 Bass kernel for: sigmoid(rowdot(tanh(x1@W.T+b), tanh(x2@W.T+b))).

Sharding: pure data-parallel over batch across 8 NeuronCores.
Per-core shapes are hardcoded (B=65536 total, 8192 rows/core, D_IN=1024,
D_PROJ=128).

Per-core dataflow (batch tile BT=512 rows):
  1. DMA load x1/x2 natural tiles [128p, 4g, 1024d] (2 MiB each, HWDGE).
  2. PE transpose-mode: x[128b,128d] blocks -> PSUM xT blocks, giving
     xT chunks [128d, 512b] (contraction dim on partitions).
  3. DVE/ACT copy PSUM -> SBUF (alternating engines to balance load).
  4. PE matmul (float32r, N=512): oT[j,b] += Wt_k.T @ xT_k, k=0..7.
  5. ACT: t = tanh(oT + bias)  (bias per-partition, PSUM -> SBUF, fused).
  6. DVE: prod = t1 * t2.
  7. PE: sim[1,512] = ones.T @ prod   (partition reduction).
  8. ACT: out = sigmoid(sim); DMA out.
"""

import numpy as np

import concourse.bacc as bacc
import concourse.mybir as mybir
import concourse.tile as tile
from concourse.bass_utils import run_bass_kernel_spmd

N_CORES = 8
B_TOTAL = 65536
BSH = B_TOTAL // N_CORES  # 8192 rows per core
D_IN = 1024
D_PROJ = 128
P = 128
BT = 512                 # batch tile (matmul moving dim)
G = BT // P              # 4 row-groups of 128 per batch tile
NBT = BSH // BT          # 16 batch tiles per core
KC = D_IN // P           # 8 contraction chunks

F32 = mybir.dt.float32
F32R = mybir.dt.float32r


def _build_module():
    nc = bacc.Bacc("TRN2", target_bir_lowering=False, debug=False)

    x1 = nc.dram_tensor("x1", [BSH, D_IN], F32R, kind="ExternalInput").ap()
    x2 = nc.dram_tensor("x2", [BSH, D_IN], F32R, kind="ExternalInput").ap()
    wt = nc.dram_tensor("wt", [D_IN, D_PROJ], F32R, kind="ExternalInput").ap()
    bias = nc.dram_tensor("bias", [P, 1], F32, kind="ExternalInput").ap()
    ident = nc.dram_tensor("ident", [P, P], F32R, kind="ExternalInput").ap()
    ones = nc.dram_tensor("ones", [P, 1], F32R, kind="ExternalInput").ap()
    out = nc.dram_tensor("out", [BSH], F32, kind="ExternalOutput").ap()

    out2d = out.rearrange("(t n) -> t n", n=BT)  # [NBT, BT]
    x1t = x1.rearrange("(t g p) d -> t p g d", p=P, g=G)  # [NBT, 128, G, D_IN]
    x2t = x2.rearrange("(t g p) d -> t p g d", p=P, g=G)

    with tile.TileContext(nc) as tc:
        with (
            tc.tile_pool(name="consts", bufs=1) as cpool,
            tc.tile_pool(name="xnat", bufs=3) as natpool,
            tc.tile_pool(name="xt", bufs=2) as xtpool,
            tc.tile_pool(name="acts", bufs=2) as apool,
            tc.tile_pool(name="outs", bufs=2) as opool_sb,
            tc.tile_pool(name="ptr", bufs=4, space="PSUM") as trpool,
            tc.tile_pool(name="po", bufs=3, space="PSUM") as opool,
            tc.tile_pool(name="psim", bufs=1, space="PSUM") as simpool,
        ):
            wt_sb = cpool.tile([P, KC, D_PROJ], F32R, tag="wt")
            nc.sync.dma_start(out=wt_sb, in_=wt.rearrange("(k p) j -> p k j", p=P))
            bias_sb = cpool.tile([P, 1], F32, tag="bias")
            nc.sync.dma_start(out=bias_sb, in_=bias)
            ident_sb = cpool.tile([P, P], F32R, tag="ident")
            nc.sync.dma_start(out=ident_sb, in_=ident)
            ones_sb = cpool.tile([P, 1], F32R, tag="ones")
            nc.sync.dma_start(out=ones_sb, in_=ones)

            for bt in range(NBT):
                xn1 = natpool.tile([P, G, D_IN], F32R, tag="xn1")
                nc.sync.dma_start(out=xn1, in_=x1t[bt])
                xn2 = natpool.tile([P, G, D_IN], F32R, tag="xn2")
                nc.sync.dma_start(out=xn2, in_=x2t[bt])

                # Transpose x tiles: xT[k][:, g*128:(g+1)*128] = x_blk(g,k).T
                # x1's matmuls are interleaved into x2's transpose stream so
                # PE has real-MM activity throughout the tile (HAM warmth)
                # and the o1 accumulation starts as early as possible.
                xt1_sb = xtpool.tile([P, KC, BT], F32R, tag="xt1")
                xt2_sb = xtpool.tile([P, KC, BT], F32R, tag="xt2")
                po1 = opool.tile([P, BT], F32, tag="po")
                po2 = opool.tile([P, BT], F32, tag="po")

                def tr_chunk(xn, xt_sb, k, eng):
                    ps = trpool.tile([P, BT], F32R, tag="tr")
                    for g in range(G):
                        nc.tensor.transpose(
                            ps[:, g * P:(g + 1) * P],
                            xn[:, g, k * P:(k + 1) * P],
                            ident_sb,
                        )
                    if eng == 0:
                        nc.vector.tensor_copy(xt_sb[:, k, :], ps)
                    else:
                        nc.scalar.copy(xt_sb[:, k, :], ps)

                def mm_chunk(po, xt_sb, k):
                    nc.tensor.matmul(
                        po,
                        wt_sb[:, k, :],
                        xt_sb[:, k, :],
                        start=(k == 0),
                        stop=(k == KC - 1),
                        skip_group_check=True,
                    )

                for k in range(KC):
                    tr_chunk(xn1, xt1_sb, k, k % 2)
                for k in range(KC):
                    tr_chunk(xn2, xt2_sb, k, (k + 1) % 2)
                    mm_chunk(po1, xt1_sb, k)
                for k in range(KC):
                    mm_chunk(po2, xt2_sb, k)

                ts = []
                for tens, po in ((0, po1), (1, po2)):
                    t_sb = apool.tile([P, BT], F32, tag=f"t{tens}")
                    nc.scalar.activation(
                        t_sb, po, mybir.ActivationFunctionType.Tanh, bias=bias_sb
                    )
                    ts.append(t_sb)

                prod = apool.tile([P, BT], F32R, tag="prod")
                nc.vector.tensor_mul(prod, ts[0], ts[1])

                psim = simpool.tile([P, BT], F32, tag="sim")
                nc.tensor.matmul(
                    psim[0:1, :],
                    ones_sb,
                    prod,
                    start=True,
                    stop=True,
                )
                sim_sb = opool_sb.tile([1, BT], F32, tag="sim_sb")
                nc.scalar.activation(
                    sim_sb, psim[0:1, :], mybir.ActivationFunctionType.Sigmoid
                )
                nc.scalar.dma_start(out=out2d[bt:bt + 1, :], in_=sim_sb)

    nc.compile()
    return nc


_NC_CACHE = None


def _get_module():
    global _NC_CACHE
    if _NC_CACHE is None:
        _NC_CACHE = _build_module()
    return _NC_CACHE


def kernel(x1, x2, W, b):
    x1 = np.ascontiguousarray(x1, dtype=np.float32)
    x2 = np.ascontiguousarray(x2, dtype=np.float32)
    wt = np.ascontiguousarray(np.asarray(W, dtype=np.float32).T)
    bias = np.ascontiguousarray(np.asarray(b, dtype=np.float32).reshape(P, 1))
    ident = np.eye(P, dtype=np.float32)
    ones = np.ones((P, 1), dtype=np.float32)

    nc = _get_module()
    in_maps = [
        {
            "x1": x1[i * BSH:(i + 1) * BSH],
            "x2": x2[i * BSH:(i + 1) * BSH],
            "wt": wt,
            "bias": bias,
            "ident": ident,
            "ones": ones,
        }
        for i in range(N_CORES)
    ]
    res = run_bass_kernel_spmd(nc, in_maps, core_ids=list(range(N_CORES)))
    return np.concatenate([res.results[i]["out"] for i in range(N_CORES)])
